# revision 16
# baseline (speedup 1.0000x reference)
"""Block-diagonal grouped conv2d (64 heads, 4->4 ch each, 3x3, pad 1) on 8 trn2 cores.

Strategy:
- Data-parallel over batch: 4 images per core, no collectives.
- Channels -> SBUF partitions, one 128-channel half per pass, half hf =
  heads [32*hf, 32*hf+32). Partition layout p = ic*32 + h_local so every
  DMA is a contiguous 32-partition slice of contiguous DRAM channels
  (channel c = ic*64 + h).
- Conv as 9 shifted matmuls accumulated in PSUM: per 3x3 offset a
  128x128 stationary matrix W[ic*32+h, oc*32+h] = w[head, oc, ic, dy, dx]
  (a permuted block-diagonal), built on the host.
- float32r matmuls (fp32 storage, 1 col/cycle on PE for N>=256).
- Rows padded to 130 cols (zero borders) so x-shifts stay in-row; strips
  of 16 rows with 1-row halo; flat 416-wide matmul chunks.
"""

import numpy as np

import concourse.bass as bass
import concourse.bacc as bacc
import concourse.mybir as mybir
from concourse.tile import TileContext
from concourse.bass_utils import run_bass_kernel_spmd

# problem shapes (hardcoded per harness contract)
B, CIN, H, W = 32, 256, 128, 128
M, CPO, CPI = 64, 4, 4
NCORES = 8
BC = B // NCORES          # images per core
R = 16                    # output rows per strip
HALO = R + 2              # input rows per strip
WP = W + 2                # padded row width
NSTRIP = H // R
CHUNK = 416               # matmul free dim: 5*416 == R*WP
NCHUNK = (R * WP) // CHUNK
FIN = HALO * WP + 2       # in-tile flat size (+1 guard elem each end)
FOUT = R * WP

F32R = mybir.dt.float32r
F32 = mybir.dt.float32

OFFS = [(dy, dx) for dy in (-1, 0, 1) for dx in (-1, 0, 1)]

_cache = {}


def _build_nc(repeat: int):
    nc = bacc.Bacc("TRN2", target_bir_lowering=False, debug=False,
                   num_devices=NCORES)
    x_d = nc.dram_tensor("x", (BC, CIN, H, W), F32R, kind="ExternalInput").ap()
    w_d = nc.dram_tensor("wstack", (18, 128, 128), F32R, kind="ExternalInput").ap()
    b_d = nc.dram_tensor("bias2", (128, 2), F32, kind="ExternalInput").ap()
    o_d = nc.dram_tensor("out", (BC, CIN, H, W), F32, kind="ExternalOutput").ap()

    with TileContext(nc) as tc:
        with tc.tile_pool(name="wpool", bufs=1) as wpool, \
             tc.tile_pool(name="xin", bufs=3) as xinp, \
             tc.tile_pool(name="xout", bufs=3) as xoutp, \
             tc.tile_pool(name="psum", bufs=4, space="PSUM") as psp:

            wsb = wpool.tile([128, 18 * 128], F32R)
            for t in range(18):
                nc.sync.dma_start(
                    out=wsb[:, t * 128:(t + 1) * 128], in_=w_d[t])
            bsb = wpool.tile([128, 2], F32)
            nc.sync.dma_start(out=bsb[:], in_=b_d)

            for rep in range(repeat):
                for b in range(BC):
                    for s in range(NSTRIP):
                        y0 = s * R
                        # valid input rows [ry0, ry1) of image; tile row 0 is y0-1
                        ry0 = max(y0 - 1, 0)
                        ry1 = min(y0 + R + 1, H)
                        r_lo = ry0 - (y0 - 1)
                        r_hi = ry1 - (y0 - 1)
                        for hf in range(2):
                            xt = xinp.tile([128, FIN], F32R, tag=f"xin{hf}")
                            nc.gpsimd.memset(xt[:].bitcast(F32), 0.0)
                            x3 = xt[:, 1:1 + HALO * WP].rearrange(
                                "p (r c) -> p r c", c=WP)
                            for i in range(CPI):
                                nc.sync.dma_start(
                                    out=x3[32 * i:32 * i + 32,
                                           r_lo:r_hi, 1:1 + W],
                                    in_=x_d[b, i * 64 + 32 * hf:
                                            i * 64 + 32 * hf + 32, ry0:ry1, :])

                            ot = xoutp.tile([128, FOUT], F32, tag=f"xout{hf}")
                            for c in range(NCHUNK):
                                c0 = c * CHUNK
                                pt = psp.tile([128, CHUNK], F32)
                                for t, (dy, dx) in enumerate(OFFS):
                                    src = 1 + c0 + WP + dy * WP + dx
                                    nc.tensor.matmul(
                                        pt[:],
                                        wsb[:, (hf * 9 + t) * 128:
                                            (hf * 9 + t + 1) * 128],
                                        xt[:, src:src + CHUNK],
                                        start=(t == 0), stop=(t == 8))
                                nc.scalar.activation(
                                    ot[:, c0:c0 + CHUNK], pt[:],
                                    mybir.ActivationFunctionType.Identity,
                                    bias=bsb[:, hf:hf + 1])
                            o3 = ot.rearrange("p (r c) -> p r c", c=WP)
                            for o in range(CPO):
                                nc.sync.dma_start(
                                    out=o_d[b, o * 64 + 32 * hf:
                                            o * 64 + 32 * hf + 32,
                                            y0:y0 + R, :],
                                    in_=o3[32 * o:32 * o + 32, :, 1:1 + W])
    nc.compile()
    return nc


def _prep_weights(weights: np.ndarray) -> np.ndarray:
    # wstack[hf*9 + t][ic*32+h, oc*32+h] = weights[32*hf+h, oc, ic, dy, dx]
    ws = np.zeros((2, 9, 128, 128), dtype=np.float32)
    wr = weights.reshape(2, 32, CPO, CPI, 3, 3)
    ar = np.arange(32)
    for t, (dy, dx) in enumerate(OFFS):
        for ic in range(CPI):
            for oc in range(CPO):
                ws[:, t, ic * 32 + ar, oc * 32 + ar] = \
                    wr[:, :, oc, ic, dy + 1, dx + 1]
    return ws.reshape(18, 128, 128)


def _prep_bias(bias: np.ndarray) -> np.ndarray:
    # bias2[oc*32+h, hf] = bias[32*hf+h, oc]
    b2 = np.zeros((128, 2), dtype=np.float32)
    br = np.asarray(bias, dtype=np.float32).reshape(2, 32, CPO)
    for oc in range(CPO):
        b2[oc * 32:oc * 32 + 32, 0] = br[0, :, oc]
        b2[oc * 32:oc * 32 + 32, 1] = br[1, :, oc]
    return b2


def _get_nc(repeat: int):
    if repeat not in _cache:
        _cache[repeat] = _build_nc(repeat)
    return _cache[repeat]


def _run(x, weights, bias, repeat=1):
    nc = _get_nc(repeat)
    ws = _prep_weights(np.asarray(weights, dtype=np.float32))
    b2 = _prep_bias(np.asarray(bias, dtype=np.float32))
    x = np.asarray(x, dtype=np.float32)
    in_maps = [
        {"x": x[c * BC:(c + 1) * BC], "wstack": ws, "bias2": b2}
        for c in range(NCORES)
    ]
    res = run_bass_kernel_spmd(nc, in_maps, core_ids=list(range(NCORES)))
    return np.concatenate([res.results[c]["out"] for c in range(NCORES)],
                          axis=0)


def kernel(x, weights, bias):
    return _run(x, weights, bias, repeat=1)


# revision 21
# speedup vs baseline: 2.0888x; 2.0888x over previous
"""Block-diagonal grouped conv2d (64 heads, 4->4 ch each, 3x3, pad 1) on 8 trn2 cores.

Strategy:
- Data-parallel over batch: 4 images per core, no collectives.
- Channels -> SBUF partitions, one 128-channel half per pass, half hf =
  heads [32*hf, 32*hf+32). Partition layout p = ic*32 + h_local so every
  DMA is a contiguous 32-partition slice of contiguous DRAM channels
  (channel c = ic*64 + h).
- Conv as 9 shifted matmuls accumulated in PSUM: per 3x3 offset a
  128x128 stationary matrix W[ic*32+h, oc*32+h] = w[head, oc, ic, dy, dx]
  (a permuted block-diagonal), built on the host.
- fp16 compute (1 col/cycle on PE, fast weight load); x is DMA'd as f32
  and cast to fp16 on the vector engine. PSUM accumulates in f32.
- Rows padded to 130 cols (zero borders) so x-shifts stay in-row; strips
  of 32 rows with 1-row halo; flat 416-wide matmul chunks.
- Input DMAs on the SP HWDGE ring, output DMAs on the ACT ring.
"""

import numpy as np

import concourse.bass as bass
import concourse.bacc as bacc
import concourse.mybir as mybir
from concourse.tile import TileContext
from concourse.bass_utils import run_bass_kernel_spmd

# problem shapes (hardcoded per harness contract)
B, CIN, H, W = 32, 256, 128, 128
M, CPO, CPI = 64, 4, 4
NCORES = 8
BC = B // NCORES          # images per core
R = 32                    # output rows per strip
HALO = R + 2              # input rows per strip
WP = W + 2                # padded row width
NSTRIP = H // R
CHUNK = 416               # matmul free dim: 10*416 == R*WP
NCHUNK = (R * WP) // CHUNK
FIN = HALO * WP + 2       # in-tile flat size (+1 guard elem each end)
FOUT = R * WP

F32 = mybir.dt.float32
FP16 = mybir.dt.float16

OFFS = [(dy, dx) for dy in (-1, 0, 1) for dx in (-1, 0, 1)]

_cache = {}


def _build_nc(repeat: int):
    nc = bacc.Bacc("TRN2", target_bir_lowering=False, debug=False,
                   num_devices=NCORES)
    x_d = nc.dram_tensor("x", (BC, CIN, H, W), F32, kind="ExternalInput").ap()
    w_d = nc.dram_tensor("wstack", (18, 128, 128), FP16,
                         kind="ExternalInput").ap()
    b_d = nc.dram_tensor("bias2", (128, 2), F32, kind="ExternalInput").ap()
    o_d = nc.dram_tensor("out", (BC, CIN, H, W), F32, kind="ExternalOutput").ap()

    with TileContext(nc) as tc:
        with tc.tile_pool(name="wpool", bufs=1) as wpool, \
             tc.tile_pool(name="xin", bufs=2) as xinp, \
             tc.tile_pool(name="xh", bufs=2) as xhp, \
             tc.tile_pool(name="xout", bufs=2) as xoutp, \
             tc.tile_pool(name="psum", bufs=4, space="PSUM") as psp:

            wsb = wpool.tile([128, 18 * 128], FP16)
            for t in range(18):
                nc.sync.dma_start(
                    out=wsb[:, t * 128:(t + 1) * 128], in_=w_d[t])
            bsb = wpool.tile([128, 2], F32)
            nc.sync.dma_start(out=bsb[:], in_=b_d)

            for rep in range(repeat):
                for b in range(BC):
                    for s in range(NSTRIP):
                        y0 = s * R
                        # valid input rows [ry0, ry1) of image; tile row 0 is y0-1
                        ry0 = max(y0 - 1, 0)
                        ry1 = min(y0 + R + 1, H)
                        r_lo = ry0 - (y0 - 1)
                        r_hi = ry1 - (y0 - 1)
                        for hf in range(2):
                            xt = xinp.tile([128, FIN], F32, tag=f"xin{hf}")
                            nc.gpsimd.memset(xt[:], 0.0)
                            x3 = xt[:, 1:1 + HALO * WP].rearrange(
                                "p (r c) -> p r c", c=WP)
                            for i in range(CPI):
                                nc.sync.dma_start(
                                    out=x3[32 * i:32 * i + 32,
                                           r_lo:r_hi, 1:1 + W],
                                    in_=x_d[b, i * 64 + 32 * hf:
                                            i * 64 + 32 * hf + 32, ry0:ry1, :])
                            xb = xhp.tile([128, FIN], FP16, tag=f"xh{hf}")
                            nc.vector.tensor_copy(xb[:], xt[:])

                            ot = xoutp.tile([128, FOUT], F32, tag=f"xout{hf}")
                            for c in range(NCHUNK):
                                c0 = c * CHUNK
                                pt = psp.tile([128, CHUNK], F32)
                                for t, (dy, dx) in enumerate(OFFS):
                                    src = 1 + c0 + WP + dy * WP + dx
                                    nc.tensor.matmul(
                                        pt[:],
                                        wsb[:, (hf * 9 + t) * 128:
                                            (hf * 9 + t + 1) * 128],
                                        xb[:, src:src + CHUNK],
                                        start=(t == 0), stop=(t == 8))
                                nc.scalar.activation(
                                    ot[:, c0:c0 + CHUNK], pt[:],
                                    mybir.ActivationFunctionType.Identity,
                                    bias=bsb[:, hf:hf + 1])
                            o3 = ot.rearrange("p (r c) -> p r c", c=WP)
                            for o in range(CPO):
                                nc.scalar.dma_start(
                                    out=o_d[b, o * 64 + 32 * hf:
                                            o * 64 + 32 * hf + 32,
                                            y0:y0 + R, :],
                                    in_=o3[32 * o:32 * o + 32, :, 1:1 + W])
    nc.compile()
    return nc


def _prep_weights(weights: np.ndarray) -> np.ndarray:
    # wstack[hf*9 + t][ic*32+h, oc*32+h] = weights[32*hf+h, oc, ic, dy, dx]
    ws = np.zeros((2, 9, 128, 128), dtype=np.float32)
    wr = weights.reshape(2, 32, CPO, CPI, 3, 3)
    ar = np.arange(32)
    for t, (dy, dx) in enumerate(OFFS):
        for ic in range(CPI):
            for oc in range(CPO):
                ws[:, t, ic * 32 + ar, oc * 32 + ar] = \
                    wr[:, :, oc, ic, dy + 1, dx + 1]
    return ws.reshape(18, 128, 128).astype(np.float16)


def _prep_bias(bias: np.ndarray) -> np.ndarray:
    # bias2[oc*32+h, hf] = bias[32*hf+h, oc]
    b2 = np.zeros((128, 2), dtype=np.float32)
    br = np.asarray(bias, dtype=np.float32).reshape(2, 32, CPO)
    for oc in range(CPO):
        b2[oc * 32:oc * 32 + 32, 0] = br[0, :, oc]
        b2[oc * 32:oc * 32 + 32, 1] = br[1, :, oc]
    return b2


def _get_nc(repeat: int):
    if repeat not in _cache:
        _cache[repeat] = _build_nc(repeat)
    return _cache[repeat]


def _run(x, weights, bias, repeat=1):
    nc = _get_nc(repeat)
    ws = _prep_weights(np.asarray(weights, dtype=np.float32))
    b2 = _prep_bias(np.asarray(bias, dtype=np.float32))
    x = np.asarray(x, dtype=np.float32)
    in_maps = [
        {"x": x[c * BC:(c + 1) * BC], "wstack": ws, "bias2": b2}
        for c in range(NCORES)
    ]
    res = run_bass_kernel_spmd(nc, in_maps, core_ids=list(range(NCORES)))
    return np.concatenate([res.results[c]["out"] for c in range(NCORES)],
                          axis=0)


def kernel(x, weights, bias):
    return _run(x, weights, bias, repeat=1)
